# revision 39
# baseline (speedup 1.0000x reference)
"""Masked dot-product attention (B=64, Lq=Lk=1024, d=64, fp32) on 8 TRN2 cores.

Strategy (slot/chunk-parallel, dual-engine exp):
  - Softmax numerator+denominator are additive over k-tiles, so batches are
    CUT into chunks of k-tiles and packed into a compile-time slot profile
    (ns[0..K-1], same on all cores, SPMD); the host sums partial outputs and
    divides.  A runtime profile search picks ns minimizing per-core tiles
    (35 = ceil(276/8) for the target data vs 37 for whole-batch dealing).
  - Scores come from fp32r matmuls per k-tile (the additive key mask is an
    extra contraction row; mask value -76 underflows exp on both paths, and
    host-padded dead tiles in underfull slots contribute exactly 0).
  - exp runs on TWO engines in parallel (the kernel is otherwise bottlenecked
    by the PE at ~854ns/tile = 4x 512-col matmuls):
      * ~54% of strips: exact ACT exp (1038ns)
      * ~46%: single DVE op (1190ns): a bf16 Schraudolph bit-trick
          j16 = int16(S*(2^7/ln2) + 16250.3); P = bitcast_bf16(j16)
        Max pointwise err ~3.6%; only k-tiles >= position 1 of multi-tile
        batches are approximated (single-tile batches always land at slot
        position 0 = exact), so end-to-end rel err is ~7.7e-3 vs the 2e-2
        gate.  Approx strips' O-matmuls run in bf16 (separate bf16 copy of
        V), exact strips stay f32r; both accumulate into the same fp32 PSUM.
  - PE stream is software-pipelined: O-matmuls lag S-matmuls by PIPE_LAG
    tiles so exp latency never stalls the PE.
  - PSUM: 3 score strips [128,1024] + 2 opsum halves [65,512] = 8 banks.
  - Per-slot output [65,1024] (numerators + denominator row) is copied
    PSUM->SBUF split across ACT (lo) and DVE (hi), then DMA'd out in one
    transfer (HWDGE charges ~625ns per DMA instruction).  Input
    DMAs are ordered first-needed-first (slot0 in three chunks so the first
    matmul starts ~3.4us in; later slots' Q/K before the bulk V transfers).
  - valid_len==0 batches are computed on the host (uniform attention =
    mean of V, exactly matching softmax of a constant).
TimelineSim (the graded cost model): 41996ns vs 50400ns baseline.
"""

import ml_dtypes
import numpy as np

import concourse.bass as bass
import concourse.mybir as mybir
import concourse.tile as tile
from concourse import bacc
from concourse.bass_utils import run_bass_kernel_spmd

N_CORES = 8
B = 64
L = 1024
D = 64
KT = L // 128
NEG = -76.0                  # additive mask; exp(-76+s) underflows both paths
LN2 = float(np.log(2.0))
EXP_A16 = 2.0**7 / LN2       # schraudolph scale (bf16 bitcast form)
EXP_B16 = 16250.3            # tuned single-term bf16 magic

F32 = mybir.dt.float32
F32R = mybir.dt.float32r
I32 = mybir.dt.int32
I16 = mybir.dt.int16
BF16 = mybir.dt.bfloat16

PIPE_LAG = 4                 # O-matmuls lag S-matmuls by this many tiles
APPROX_FRAC = 0.457          # fraction of strips on the schraudolph path

_prog_cache = {}


def _approx_set(tiles, NT):
    """Pick ~APPROX_FRAC of tiles (never slot position 0, never the final
    tile: its exact-exp chain is shorter on the critical tail) spread
    evenly."""
    target = int(round(NT * APPROX_FRAC))
    appr = set()
    stride = NT / max(target, 1)
    next_at = 1.0
    for idx, (s, kt) in enumerate(tiles):
        if (len(appr) < target and kt >= 1 and idx < NT - 1
                and idx + 1e-6 >= next_at):
            appr.add((s, kt))
            next_at += stride
    return appr


# ---------------------------------------------------------------- program ---

def _build_program(ns):
    """ns: per-slot k-tile counts (desc tuple). Same program on all cores."""
    K = len(ns)
    NT = sum(ns)
    qoff = []
    off = 0
    for s in range(K):
        qoff.append(off)
        off += 128 * ns[s] + 1024
    QCOLS = off
    voff = []
    off = 0
    for s in range(K):
        voff.append(off)
        off += ns[s]

    nc = bacc.Bacc("TRN2", target_bir_lowering=False, debug=False,
                   num_devices=N_CORES)
    qkt_d = nc.dram_tensor("qkt", [D + 1, QCOLS], F32R, kind="ExternalInput")
    vp_d = nc.dram_tensor("vp", [128, NT, D + 1], F32R, kind="ExternalInput")
    vpb_d = nc.dram_tensor("vpb", [128, NT, D + 1], BF16, kind="ExternalInput")
    o_d = nc.dram_tensor("o", [D + 1, K * 1024], F32, kind="ExternalOutput")

    with tile.TileContext(nc) as tc:
        with (
            tc.tile_pool(name="qk", bufs=1) as qk_pool,
            tc.tile_pool(name="vpp", bufs=1) as vp_pool,
            tc.tile_pool(name="pt", bufs=PIPE_LAG + 3) as pt_pool,
            tc.tile_pool(name="osb", bufs=1) as osb_pool,
            tc.tile_pool(name="sp", bufs=3, space="PSUM") as sp_pool,
            tc.tile_pool(name="op", bufs=1, space="PSUM") as op_pool,
        ):
            # ---- input DMAs, first-needed first ----
            qs = [qk_pool.tile([D + 1, 128 * ns[s] + 1024], F32R,
                               tag=f"qkt{s}", bufs=1, name=f"qkt{s}")
                  for s in range(K)]
            vs = vp_pool.tile([128, NT, D + 1], F32R, tag="vp", bufs=1)
            vsb = vp_pool.tile([128, NT, D + 1], BF16, tag="vpb", bufs=1)
            w0 = 128 + 512                      # slot0 head: ktm0 + half of qt
            nc.sync.dma_start(qs[0][:, :w0], qkt_d[:, :w0])
            e0 = 128 * ns[0] + 1024
            # qt_hi + ktm1,ktm2 next: strips 0-2 runnable off two chunks
            w1 = min(128 + 1024 + 2 * 128, e0)
            nc.sync.dma_start(qs[0][:, w0:w1], qkt_d[:, w0:w1])
            if e0 > w1:
                nc.sync.dma_start(qs[0][:, w1:e0], qkt_d[:, w1:e0])
            nc.sync.dma_start(vs[:, :ns[0], :], vp_d[:, :ns[0], :])
            nc.sync.dma_start(vsb[:, :ns[0], :], vpb_d[:, :ns[0], :])
            # qkt of the next two slots BEFORE the bulk vp transfers: their
            # S-matmuls come up sooner than any vp use
            for s in range(1, min(3, K)):
                w = 128 * ns[s] + 1024
                nc.sync.dma_start(qs[s][:], qkt_d[:, qoff[s]:qoff[s] + w])
            if K > 1:
                nc.sync.dma_start(vs[:, ns[0]:, :], vp_d[:, ns[0]:, :])
                nc.sync.dma_start(vsb[:, ns[0]:, :], vpb_d[:, ns[0]:, :])
            for s in range(3, K):
                w = 128 * ns[s] + 1024
                nc.sync.dma_start(qs[s][:], qkt_d[:, qoff[s]:qoff[s] + w])

            def ktm_sl(s, kt):
                if kt == 0:
                    return qs[s][:, :128]
                o_ = 128 + 1024 + (kt - 1) * 128
                return qs[s][:, o_:o_ + 128]

            def qt_sl(s, h):
                return qs[s][:, 128 + h * 512:128 + (h + 1) * 512]

            # ---- flat tile schedule with O-lag pipeline ----
            tiles = [(s, kt) for s in range(K) for kt in range(ns[s])]
            appr = _approx_set(tiles, NT)
            pt_of = {}
            op_of = {}

            def emit_S(idx):
                s, kt = tiles[idx]
                sp = sp_pool.tile([128, 1024], F32, tag="s",
                                  name=f"sp{idx}")
                nc.tensor.matmul(sp[:, :512], ktm_sl(s, kt), qt_sl(s, 0),
                                 start=True, stop=True)
                nc.tensor.matmul(sp[:, 512:], ktm_sl(s, kt), qt_sl(s, 1),
                                 start=True, stop=True)
                pt_of[idx] = (sp, None)

            def emit_exp(idx):
                s, kt = tiles[idx]
                sp, _ = pt_of[idx]
                if (s, kt) not in appr:
                    pt = pt_pool.tile([128, 1024], F32R, tag="pte",
                                      bufs=PIPE_LAG + 3, name=f"pte{idx}")
                    nc.scalar.activation(pt[:], sp[:],
                                         mybir.ActivationFunctionType.Exp)
                else:
                    pt = pt_pool.tile([128, 1024], BF16, tag="pta",
                                      bufs=PIPE_LAG + 3, name=f"pta{idx}")
                    nc.vector.tensor_scalar(pt[:].bitcast(I16), sp[:],
                                            EXP_A16, EXP_B16,
                                            mybir.AluOpType.mult,
                                            mybir.AluOpType.add)
                pt_of[idx] = (sp, pt)

            op_of = {}

            def emit_O(idx):
                s, kt = tiles[idx]
                if kt == 0:
                    opl_t = op_pool.tile([D + 1, 512], F32, tag="opl",
                                         name=f"opl{s}")
                    oph_t = op_pool.tile([D + 1, 512], F32, tag="oph",
                                         name=f"oph{s}")
                    op_of[s] = (opl_t, oph_t)
                opl, oph = op_of[s]
                vsrc = vsb if (s, kt) in appr else vs
                vt = vsrc[:, voff[s] + kt, :]
                _, pt = pt_of.pop(idx)
                st, fin = (kt == 0), (kt == ns[s] - 1)
                nc.tensor.matmul(opl[:], vt, pt[:, :512], start=st, stop=fin)
                nc.tensor.matmul(oph[:], vt, pt[:, 512:], start=st, stop=fin)
                if fin:
                    ob = s * 1024
                    ot = osb_pool.tile([D + 1, 1024], F32, tag=f"os{s}",
                                       bufs=1, name=f"os{s}")
                    nc.scalar.copy(ot[:, :512], opl[:])
                    nc.vector.tensor_copy(ot[:, 512:], oph[:])
                    nc.sync.dma_start(o_d[:, ob:ob + 1024], ot[:])

            n = len(tiles)
            # per-tile O gate: exact exp latency ~1.2us -> lag 2; approx
            # (DVE affine) needs the full PIPE_LAG.  Monotone so per-slot
            # start/stop accumulation order is preserved.
            gates = []
            g = -1
            for idx2, (s2, kt2) in enumerate(tiles):
                lag = PIPE_LAG if (s2, kt2) in appr else 2
                g = max(g + 1, idx2 + lag)
                gates.append(g)
            oq = 0
            i = 0
            while i < n or oq < n:
                if i < n:
                    emit_S(i)
                    emit_exp(i)
                while oq < n and gates[oq] <= i:
                    emit_O(oq)
                    oq += 1
                i += 1

    nc.compile()
    return nc


def get_program(ns):
    ns = tuple(ns)
    if ns not in _prog_cache:
        _prog_cache[ns] = _build_program(ns)
    return _prog_cache[ns]


# ------------------------------------------------------------- scheduling ---

def _partitions(n, k, mx):
    if k == 0:
        if n == 0:
            yield ()
        return
    for v in range(min(mx, n - (k - 1)), 0, -1):
        for rest in _partitions(n - v, k - 1, v):
            yield (v,) + rest


def _try_profile(ns, sizes):
    """Greedy cut-and-fill: capacities = 8 copies of each ns entry (desc).
    sizes: list of (batch_id, nact). Returns list per instance of
    (slot_idx, batch_id, start_tile, ntiles) or None."""
    caps = []
    for si, v in enumerate(ns):
        caps += [(v, si)] * 8
    caps.sort(key=lambda t: -t[0])
    total_cap = sum(c for c, _ in caps)
    T = sum(na for _, na in sizes)
    if total_cap < T:
        return None
    waste_budget = total_cap - T
    rem = sorted([[na, bi, 0] for bi, na in sizes], reverse=True)
    out = []
    waste = 0
    for cap, si in caps:
        alive = [r for r in rem if r[0] > 0]
        if not alive:
            return None
        exact = next((r for r in alive if r[0] == cap), None)
        if exact is not None:
            r = exact
            take = cap
        else:
            r = max(alive)
            take = min(cap, r[0])
            if r[0] < cap:
                waste += cap - r[0]
                if waste > waste_budget:
                    return None
        out.append((si, r[1], r[2], take))
        r[0] -= take
        r[2] += take
    if any(r[0] > 0 for r in rem):
        return None
    return out


def _search_profile(nacts):
    """nacts: list of (batch_id, nact). Returns (ns, chunks)."""
    T = sum(na for _, na in nacts)
    mx = max(na for _, na in nacts)
    lo = max((T + N_CORES - 1) // N_CORES, mx)
    for N in range(lo, T + 1):
        for K in range(8, 13):
            for ns in _partitions(N, K, mx):
                got = _try_profile(ns, nacts)
                if got is not None:
                    return ns, got
    raise RuntimeError("no feasible profile")


# ------------------------------------------------------------------ host ---

def _pack_core(ns, chunks_c, q, k, v, vl):
    """chunks_c: list of (slot, batch, start_tile, ntiles) for one core."""
    K = len(ns)
    NT = sum(ns)
    QCOLS = sum(128 * n + 1024 for n in ns)
    qkt = np.zeros((D + 1, QCOLS), np.float32)
    vp = np.zeros((128, NT, D + 1), np.float32)
    by_slot = {c[0]: c for c in chunks_c}
    qo = 0
    vo = 0
    iota = np.arange(128)
    scale = np.float32(1.0 / np.sqrt(D))
    for s in range(K):
        nss = ns[s]
        ch = by_slot.get(s)
        for j in range(nss):
            cols = slice(qo, qo + 128) if j == 0 else \
                slice(qo + 128 + 1024 + (j - 1) * 128,
                      qo + 128 + 1024 + j * 128)
            if ch is not None and j < ch[3]:
                b, kt = ch[1], ch[2] + j
                qkt[:D, cols] = k[b, kt * 128:(kt + 1) * 128].T
                kabs = kt * 128 + iota
                qkt[D, cols] = np.where(kabs < vl[b], 0.0, np.float32(NEG))
                vp[:, vo + j, :D] = v[b, kt * 128:(kt + 1) * 128]
                vp[:, vo + j, D] = 1.0
            else:
                qkt[D, cols] = np.float32(NEG)               # dead padding
        if ch is not None:
            b = ch[1]
            qt = qkt[:, qo + 128:qo + 128 + 1024]
            qt[:D] = q[b].T * scale
            qt[D] = 1.0
        qo += 128 * nss + 1024
        vo += nss
    return qkt, vp


def kernel(queries, keys, values, valid_lens):
    q = np.asarray(queries, np.float32)
    k = np.asarray(keys, np.float32)
    v = np.asarray(values, np.float32)
    vl = np.asarray(valid_lens).astype(np.int64)

    out = np.empty((B, L, D), np.float32)

    # valid_len == 0 -> uniform attention over all keys (softmax of const).
    for b in np.nonzero(vl == 0)[0]:
        out[b] = v[b].mean(0, keepdims=True)

    live = [int(b) for b in range(B) if vl[b] > 0]
    nacts = [(b, int(-(-vl[b] // 128))) for b in live]
    ns0, chunks0 = _search_profile(nacts)
    # process slots smallest-first: slot-boundary overheads overlap the
    # DMA-bound fill phase, big slots stream gap-free afterwards
    order = sorted(range(len(ns0)), key=lambda s: -ns0[s])
    remap = {old: new for new, old in enumerate(order)}
    ns = tuple(ns0[s] for s in order)
    chunks = [(remap[si], bi, t0, nt) for (si, bi, t0, nt) in chunks0]
    nc = get_program(ns)

    # deal instances of each slot to cores round-robin
    percore = [[] for _ in range(N_CORES)]
    slot_seen = {}
    for (si, bi, t0, nt) in chunks:
        c = slot_seen.get(si, 0)
        slot_seen[si] = c + 1
        percore[c].append((si, bi, t0, nt))

    in_maps = []
    for c in range(N_CORES):
        qkt, vp = _pack_core(ns, percore[c], q, k, v, vl)
        in_maps.append({"qkt": qkt, "vp": np.ascontiguousarray(vp),
                        "vpb": vp.astype(ml_dtypes.bfloat16)})

    res = None
    for attempt in range(3):
        try:
            res = run_bass_kernel_spmd(nc, in_maps, list(range(N_CORES)))
            break
        except Exception:
            if attempt == 2:
                raise
            import time as _time
            _time.sleep(2.0)
            try:
                import jax
                jax.clear_caches()
            except Exception:
                pass

    # host combine: sum partial numerators/denominators per batch, divide
    num = np.zeros((B, D, L), np.float32)
    den = np.zeros((B, 1, L), np.float32)
    for c in range(N_CORES):
        o = res.results[c]["o"]          # [65, K*1024]
        for (si, bi, t0, nt) in percore[c]:
            blk = o[:, si * 1024:(si + 1) * 1024]
            num[bi] += blk[:D]
            den[bi] += blk[D:D + 1]
    live_mask = vl > 0
    out[live_mask] = (num[live_mask] / den[live_mask]).transpose(0, 2, 1)
    return out


# revision 41
# speedup vs baseline: 1.0148x; 1.0148x over previous
"""Masked dot-product attention (B=64, Lq=Lk=1024, d=64, fp32) on 8 TRN2 cores.

Strategy (slot/chunk-parallel, dual-engine exp):
  - Softmax numerator+denominator are additive over k-tiles, so batches are
    CUT into chunks of k-tiles and packed into a compile-time slot profile
    (ns[0..K-1], same on all cores, SPMD); the host sums partial outputs and
    divides.  A runtime profile search picks ns minimizing per-core tiles
    (35 = ceil(276/8) for the target data vs 37 for whole-batch dealing).
  - Scores come from fp32r matmuls per k-tile (the additive key mask is an
    extra contraction row; mask value -76 underflows exp on both paths, and
    host-padded dead tiles in underfull slots contribute exactly 0).
  - exp runs on TWO engines in parallel (the kernel is otherwise bottlenecked
    by the PE at ~854ns/tile = 4x 512-col matmuls):
      * ~54% of strips: exact ACT exp (1038ns)
      * ~46%: single DVE op (1190ns): a bf16 Schraudolph bit-trick
          j16 = int16(S*(2^7/ln2) + 16250.3); P = bitcast_bf16(j16)
        Max pointwise err ~3.6%; only k-tiles >= position 1 of multi-tile
        batches are approximated (single-tile batches always land at slot
        position 0 = exact), so end-to-end rel err is ~7.7e-3 vs the 2e-2
        gate.  Approx strips' O-matmuls run in bf16 (separate bf16 copy of
        V), exact strips stay f32r; both accumulate into the same fp32 PSUM.
  - PE stream is software-pipelined: O-matmuls lag S-matmuls by PIPE_LAG
    tiles so exp latency never stalls the PE.
  - PSUM: 3 score strips [128,1024] + 2 opsum halves [65,512] = 8 banks.
  - Per-slot output [65,1024] (numerators + denominator row) is copied
    PSUM->SBUF split across ACT (lo) and DVE (hi), then DMA'd out in one
    transfer (HWDGE charges ~625ns per DMA instruction).  Input
    DMAs are ordered first-needed-first (slot0 in three chunks so the first
    matmul starts ~3.4us in; later slots' Q/K before the bulk V transfers).
  - valid_len==0 batches are computed on the host (uniform attention =
    mean of V, exactly matching softmax of a constant).
TimelineSim (the graded cost model): 41996ns vs 50400ns baseline.
"""

import ml_dtypes
import numpy as np

import concourse.bass as bass
import concourse.mybir as mybir
import concourse.tile as tile
from concourse import bacc
from concourse.bass_utils import run_bass_kernel_spmd

N_CORES = 8
B = 64
L = 1024
D = 64
KT = L // 128
NEG = -76.0                  # additive mask; exp(-76+s) underflows both paths
LN2 = float(np.log(2.0))
EXP_A16 = 2.0**7 / LN2       # schraudolph scale (bf16 bitcast form)
EXP_B16 = 16250.3            # tuned single-term bf16 magic

F32 = mybir.dt.float32
F32R = mybir.dt.float32r
I32 = mybir.dt.int32
I16 = mybir.dt.int16
BF16 = mybir.dt.bfloat16

PIPE_LAG = 4                 # O-matmuls lag S-matmuls by this many tiles
APPROX_FRAC = 0.457          # fraction of strips on the schraudolph path

_prog_cache = {}


def _approx_set(tiles, NT):
    """Pick ~APPROX_FRAC of tiles (never slot position 0, never the final
    tile: its exact-exp chain is shorter on the critical tail) spread
    evenly."""
    target = int(round(NT * APPROX_FRAC))
    appr = set()
    stride = NT / max(target, 1)
    next_at = 1.0
    for idx, (s, kt) in enumerate(tiles):
        if (len(appr) < target and kt >= 1 and idx < NT - 1
                and idx + 1e-6 >= next_at):
            appr.add((s, kt))
            next_at += stride
    return appr


# ---------------------------------------------------------------- program ---

def _build_program(ns):
    """ns: per-slot k-tile counts (desc tuple). Same program on all cores."""
    K = len(ns)
    NT = sum(ns)
    qoff = []
    off = 0
    for s in range(K):
        qoff.append(off)
        off += 128 * ns[s] + 1024
    QCOLS = off
    voff = []
    off = 0
    for s in range(K):
        voff.append(off)
        off += ns[s]

    nc = bacc.Bacc("TRN2", target_bir_lowering=False, debug=False,
                   num_devices=N_CORES)
    qkt_d = nc.dram_tensor("qkt", [D + 1, QCOLS], F32R, kind="ExternalInput")
    vp_d = nc.dram_tensor("vp", [128, NT, D + 1], F32R, kind="ExternalInput")
    vpb_d = nc.dram_tensor("vpb", [128, NT, D + 1], BF16, kind="ExternalInput")
    o_d = nc.dram_tensor("o", [D + 1, K * 1024], F32, kind="ExternalOutput")

    with tile.TileContext(nc) as tc:
        with (
            tc.tile_pool(name="qk", bufs=1) as qk_pool,
            tc.tile_pool(name="vpp", bufs=1) as vp_pool,
            tc.tile_pool(name="pt", bufs=PIPE_LAG + 3) as pt_pool,
            tc.tile_pool(name="osb", bufs=1) as osb_pool,
            tc.tile_pool(name="sp", bufs=3, space="PSUM") as sp_pool,
            tc.tile_pool(name="op", bufs=1, space="PSUM") as op_pool,
        ):
            # ---- input DMAs, first-needed first ----
            qs = [qk_pool.tile([D + 1, 128 * ns[s] + 1024], F32R,
                               tag=f"qkt{s}", bufs=1, name=f"qkt{s}")
                  for s in range(K)]
            vs = vp_pool.tile([128, NT, D + 1], F32R, tag="vp", bufs=1)
            vsb = vp_pool.tile([128, NT, D + 1], BF16, tag="vpb", bufs=1)
            w0 = 128 + 512                      # slot0 head: ktm0 + half of qt
            nc.sync.dma_start(qs[0][:, :w0], qkt_d[:, :w0])
            e0 = 128 * ns[0] + 1024
            # qt_hi + ktm1,ktm2 next: strips 0-2 runnable off two chunks
            w1 = min(128 + 1024 + 2 * 128, e0)
            nc.sync.dma_start(qs[0][:, w0:w1], qkt_d[:, w0:w1])
            if e0 > w1:
                nc.sync.dma_start(qs[0][:, w1:e0], qkt_d[:, w1:e0])
            nc.sync.dma_start(vs[:, :ns[0], :], vp_d[:, :ns[0], :])
            nc.sync.dma_start(vsb[:, :ns[0], :], vpb_d[:, :ns[0], :])
            # qkt of the next two slots BEFORE the bulk vp transfers: their
            # S-matmuls come up sooner than any vp use
            for s in range(1, min(3, K)):
                w = 128 * ns[s] + 1024
                nc.sync.dma_start(qs[s][:], qkt_d[:, qoff[s]:qoff[s] + w])
            if K > 1:
                nc.sync.dma_start(vs[:, ns[0]:, :], vp_d[:, ns[0]:, :])
                nc.sync.dma_start(vsb[:, ns[0]:, :], vpb_d[:, ns[0]:, :])
            for s in range(3, K):
                w = 128 * ns[s] + 1024
                nc.sync.dma_start(qs[s][:], qkt_d[:, qoff[s]:qoff[s] + w])

            def ktm_sl(s, kt):
                if kt == 0:
                    return qs[s][:, :128]
                o_ = 128 + 1024 + (kt - 1) * 128
                return qs[s][:, o_:o_ + 128]

            def qt_sl(s, h):
                return qs[s][:, 128 + h * 512:128 + (h + 1) * 512]

            # ---- flat tile schedule with O-lag pipeline ----
            tiles = [(s, kt) for s in range(K) for kt in range(ns[s])]
            appr = _approx_set(tiles, NT)
            flat_start = {}
            for i2, (s2, kt2) in enumerate(tiles):
                if kt2 == 0:
                    flat_start[s2] = i2
            pt_of = {}
            op_of = {}

            def emit_S(idx):
                s, kt = tiles[idx]
                sp = sp_pool.tile([128, 1024], F32, tag="s",
                                  name=f"sp{idx}")
                nc.tensor.matmul(sp[:, :512], ktm_sl(s, kt), qt_sl(s, 0),
                                 start=True, stop=True)
                nc.tensor.matmul(sp[:, 512:], ktm_sl(s, kt), qt_sl(s, 1),
                                 start=True, stop=True)
                pt_of[idx] = (sp, None)

            def emit_exp(idx):
                s, kt = tiles[idx]
                sp, _ = pt_of[idx]
                if (s, kt) not in appr:
                    pt = pt_pool.tile([128, 1024], F32R, tag="pte",
                                      bufs=PIPE_LAG + 3, name=f"pte{idx}")
                    nc.scalar.activation(pt[:], sp[:],
                                         mybir.ActivationFunctionType.Exp)
                else:
                    pt = pt_pool.tile([128, 1024], BF16, tag="pta",
                                      bufs=PIPE_LAG + 3, name=f"pta{idx}")
                    nc.vector.tensor_scalar(pt[:].bitcast(I16), sp[:],
                                            EXP_A16, EXP_B16,
                                            mybir.AluOpType.mult,
                                            mybir.AluOpType.add)
                pt_of[idx] = (sp, pt)

            op_of = {}

            def emit_O(idx):
                s, kt = tiles[idx]
                if kt == 0:
                    # strip-hosting is only safe for slots whose first O is
                    # emitted after the final S allocation, else it steals a
                    # live strip buffer from the S rotation
                    if flat_start[s] >= NT - PIPE_LAG - 1:
                        # the S-strip rotation is dead by now: host this
                        # slot's opsum in a strip buffer (both halves), so
                        # the tail slots' O-chains don't serialize on the
                        # single-buffered op pool via copy WARs
                        opf = sp_pool.tile([128, 1024], F32, tag="s",
                                           name=f"opf{s}")
                        op_of[s] = (opf[:D + 1, :512], opf[:D + 1, 512:])
                    else:
                        opl_t = op_pool.tile([D + 1, 512], F32, tag="opl",
                                             name=f"opl{s}")
                        oph_t = op_pool.tile([D + 1, 512], F32, tag="oph",
                                             name=f"oph{s}")
                        op_of[s] = (opl_t, oph_t)
                opl, oph = op_of[s]
                vsrc = vsb if (s, kt) in appr else vs
                vt = vsrc[:, voff[s] + kt, :]
                _, pt = pt_of.pop(idx)
                st, fin = (kt == 0), (kt == ns[s] - 1)
                nc.tensor.matmul(opl[:], vt, pt[:, :512], start=st, stop=fin)
                nc.tensor.matmul(oph[:], vt, pt[:, 512:], start=st, stop=fin)
                if fin:
                    ob = s * 1024
                    ot = osb_pool.tile([D + 1, 1024], F32, tag=f"os{s}",
                                       bufs=1, name=f"os{s}")
                    nc.scalar.copy(ot[:, :512], opl[:])
                    nc.vector.tensor_copy(ot[:, 512:], oph[:])
                    nc.sync.dma_start(o_d[:, ob:ob + 1024], ot[:])

            n = len(tiles)
            # per-tile O gate: exact exp latency ~1.2us -> lag 2; approx
            # (DVE affine) needs the full PIPE_LAG.  Monotone so per-slot
            # start/stop accumulation order is preserved.
            gates = []
            g = -1
            for idx2, (s2, kt2) in enumerate(tiles):
                lag = PIPE_LAG if (s2, kt2) in appr else 2
                g = max(g + 1, idx2 + lag)
                gates.append(g)
            oq = 0
            i = 0
            while i < n or oq < n:
                if i < n:
                    emit_S(i)
                    emit_exp(i)
                while oq < n and gates[oq] <= i:
                    emit_O(oq)
                    oq += 1
                i += 1

    nc.compile()
    return nc


def get_program(ns):
    ns = tuple(ns)
    if ns not in _prog_cache:
        _prog_cache[ns] = _build_program(ns)
    return _prog_cache[ns]


# ------------------------------------------------------------- scheduling ---

def _partitions(n, k, mx):
    if k == 0:
        if n == 0:
            yield ()
        return
    for v in range(min(mx, n - (k - 1)), 0, -1):
        for rest in _partitions(n - v, k - 1, v):
            yield (v,) + rest


def _try_profile(ns, sizes):
    """Greedy cut-and-fill: capacities = 8 copies of each ns entry (desc).
    sizes: list of (batch_id, nact). Returns list per instance of
    (slot_idx, batch_id, start_tile, ntiles) or None."""
    caps = []
    for si, v in enumerate(ns):
        caps += [(v, si)] * 8
    caps.sort(key=lambda t: -t[0])
    total_cap = sum(c for c, _ in caps)
    T = sum(na for _, na in sizes)
    if total_cap < T:
        return None
    waste_budget = total_cap - T
    rem = sorted([[na, bi, 0] for bi, na in sizes], reverse=True)
    out = []
    waste = 0
    for cap, si in caps:
        alive = [r for r in rem if r[0] > 0]
        if not alive:
            return None
        exact = next((r for r in alive if r[0] == cap), None)
        if exact is not None:
            r = exact
            take = cap
        else:
            r = max(alive)
            take = min(cap, r[0])
            if r[0] < cap:
                waste += cap - r[0]
                if waste > waste_budget:
                    return None
        out.append((si, r[1], r[2], take))
        r[0] -= take
        r[2] += take
    if any(r[0] > 0 for r in rem):
        return None
    return out


def _search_profile(nacts):
    """nacts: list of (batch_id, nact). Returns (ns, chunks)."""
    T = sum(na for _, na in nacts)
    mx = max(na for _, na in nacts)
    lo = max((T + N_CORES - 1) // N_CORES, mx)
    for N in range(lo, T + 1):
        for K in range(8, 13):
            for ns in _partitions(N, K, mx):
                got = _try_profile(ns, nacts)
                if got is not None:
                    return ns, got
    raise RuntimeError("no feasible profile")


# ------------------------------------------------------------------ host ---

def _pack_core(ns, chunks_c, q, k, v, vl):
    """chunks_c: list of (slot, batch, start_tile, ntiles) for one core."""
    K = len(ns)
    NT = sum(ns)
    QCOLS = sum(128 * n + 1024 for n in ns)
    qkt = np.zeros((D + 1, QCOLS), np.float32)
    vp = np.zeros((128, NT, D + 1), np.float32)
    by_slot = {c[0]: c for c in chunks_c}
    qo = 0
    vo = 0
    iota = np.arange(128)
    scale = np.float32(1.0 / np.sqrt(D))
    for s in range(K):
        nss = ns[s]
        ch = by_slot.get(s)
        for j in range(nss):
            cols = slice(qo, qo + 128) if j == 0 else \
                slice(qo + 128 + 1024 + (j - 1) * 128,
                      qo + 128 + 1024 + j * 128)
            if ch is not None and j < ch[3]:
                b, kt = ch[1], ch[2] + j
                qkt[:D, cols] = k[b, kt * 128:(kt + 1) * 128].T
                kabs = kt * 128 + iota
                qkt[D, cols] = np.where(kabs < vl[b], 0.0, np.float32(NEG))
                vp[:, vo + j, :D] = v[b, kt * 128:(kt + 1) * 128]
                vp[:, vo + j, D] = 1.0
            else:
                qkt[D, cols] = np.float32(NEG)               # dead padding
        if ch is not None:
            b = ch[1]
            qt = qkt[:, qo + 128:qo + 128 + 1024]
            qt[:D] = q[b].T * scale
            qt[D] = 1.0
        qo += 128 * nss + 1024
        vo += nss
    return qkt, vp


def kernel(queries, keys, values, valid_lens):
    q = np.asarray(queries, np.float32)
    k = np.asarray(keys, np.float32)
    v = np.asarray(values, np.float32)
    vl = np.asarray(valid_lens).astype(np.int64)

    out = np.empty((B, L, D), np.float32)

    # valid_len == 0 -> uniform attention over all keys (softmax of const).
    for b in np.nonzero(vl == 0)[0]:
        out[b] = v[b].mean(0, keepdims=True)

    live = [int(b) for b in range(B) if vl[b] > 0]
    nacts = [(b, int(-(-vl[b] // 128))) for b in live]
    ns0, chunks0 = _search_profile(nacts)
    # process slots smallest-first: slot-boundary overheads overlap the
    # DMA-bound fill phase, big slots stream gap-free afterwards
    order = sorted(range(len(ns0)), key=lambda s: -ns0[s])
    remap = {old: new for new, old in enumerate(order)}
    ns = tuple(ns0[s] for s in order)
    chunks = [(remap[si], bi, t0, nt) for (si, bi, t0, nt) in chunks0]
    nc = get_program(ns)

    # deal instances of each slot to cores round-robin
    percore = [[] for _ in range(N_CORES)]
    slot_seen = {}
    for (si, bi, t0, nt) in chunks:
        c = slot_seen.get(si, 0)
        slot_seen[si] = c + 1
        percore[c].append((si, bi, t0, nt))

    in_maps = []
    for c in range(N_CORES):
        qkt, vp = _pack_core(ns, percore[c], q, k, v, vl)
        in_maps.append({"qkt": qkt, "vp": np.ascontiguousarray(vp),
                        "vpb": vp.astype(ml_dtypes.bfloat16)})

    res = None
    for attempt in range(3):
        try:
            res = run_bass_kernel_spmd(nc, in_maps, list(range(N_CORES)))
            break
        except Exception:
            if attempt == 2:
                raise
            import time as _time
            _time.sleep(2.0)
            try:
                import jax
                jax.clear_caches()
            except Exception:
                pass

    # host combine: sum partial numerators/denominators per batch, divide
    num = np.zeros((B, D, L), np.float32)
    den = np.zeros((B, 1, L), np.float32)
    for c in range(N_CORES):
        o = res.results[c]["o"]          # [65, K*1024]
        for (si, bi, t0, nt) in percore[c]:
            blk = o[:, si * 1024:(si + 1) * 1024]
            num[bi] += blk[:D]
            den[bi] += blk[D:D + 1]
    live_mask = vl > 0
    out[live_mask] = (num[live_mask] / den[live_mask]).transpose(0, 2, 1)
    return out
